# revision 4
# baseline (speedup 1.0000x reference)
"""Trainium2 Bass kernel for nn_HeatmapBatch.

Reference computes: one-hot delta (value 10.0) per (batch, keypoint) at
integer coords (r, c) in a 256x256 image, then depthwise-convolves with a
shared 9x9 kernel.  Since each image holds exactly one delta, the output is
zeros everywhere except a 9x9 patch of 10*kernel2d[::-1,::-1] (XLA conv is
cross-correlation) centred at (r, c), clipped at the borders.

Device strategy (data-parallel over batch, 8 cores x 8 batches = 168
images per core):

  Each image gets a fully padded 264x264 canvas (4 pad rows/cols on every
  side), so every patch -- clipped or not -- is in-bounds: no clip
  handling, no dump zone (the host strips the padding on assemble).

  The gpsimd indirect-DMA scatter writes, per SBUF partition, ONE
  contiguous run (= the partition's whole source payload) at that
  partition's index (hardware-verified semantics; extra offset-AP columns
  are ignored for addressing).  A patch is therefore written as a small
  number of contiguous multi-row spans whose inter-row gaps are zeros
  (overwriting pad/zero cells with zeros is harmless):

    - mode "spans2": whole 9-row spans (2121 elems), 2 calls (128+40
      partitions), 1.43 MB written/core.
    - mode "rows3": 3-row spans (537 elems), 4 calls x 126 partitions
      (run s of patch p sits at partition q with q%3 == s, so one shared
      [126, 537] value buffer serves all 4 calls), 1.08 MB written/core.

  The value buffer is composed without any vector-compute stage: host
  pre-scales (10*flip), a strided sync-DMA drops the rows into their
  span slots, and a vector memset zeroes the gaps (disjoint regions, no
  ordering).  gpsimd issues the index DMA itself (SWDGE) to skip one
  cross-engine handoff.  The runtime hands kernels pre-zeroed
  ExternalOutput buffers (documented contract in bass_utils/bass2jax),
  so the kernel only scatters; a zero-fill variant covers the contract
  ever failing (detected by output sampling), and the previous session's
  hardware-validated kernel remains as a final fallback.
Host does sharding/layout prep and the final gather/strip of the padding.
"""

import numpy as np


def _ensure_axon_hooks():
    """bass_utils imports antenv.axon_hooks when tracing is requested (e.g.
    BASS_TRACE=1 in the environment); some images lack that module.  Provide
    it best-effort so a tracing harness degrades gracefully instead of
    crashing.  Never raises."""
    try:
        import antenv.axon_hooks  # noqa: F401
        return
    except Exception:
        pass
    try:
        import sys
        import types

        import antenv

        mod = types.ModuleType("antenv.axon_hooks")
        _state = {"hook": None}
        mod.set_axon_ntff_profile_hook = lambda h: _state.__setitem__("hook", h)
        mod.get_axon_ntff_profile_hook = lambda: _state["hook"]
        sys.modules["antenv.axon_hooks"] = mod
        antenv.axon_hooks = mod
        try:
            from trn_agent_boot.trn_boot import _ntff_profile_via_ctypes

            mod.set_axon_ntff_profile_hook(
                _ntff_profile_via_ctypes("/opt/axon/libaxon_pjrt.so")
            )
        except Exception:
            pass
    except Exception:
        pass


_ensure_axon_hooks()

B, KP, H = 64, 21, 256
KS, PAD = 9, 4
NCORES = 8
BLOC = B // NCORES          # 8 batches per core
NPTS = BLOC * KP            # 168 images per core
WPAD = H + 2 * PAD          # 264 padded columns
HP = H + 2 * PAD            # 264 padded rows per image
OROWS = NPTS * HP           # 44352 output rows per core

# mode -> (partitions, idx cols, rows per span, span length, calls)
# calls: list of (n_partitions, idx column) per indirect DMA
_MODES = {
    "spans2": (128, 2, 9, 8 * WPAD + KS, [(128, 0), (40, 1)]),
    "rows3": (126, 4, 3, 2 * WPAD + KS, [(126, 0), (126, 1), (126, 2), (126, 3)]),
}

_NC_CACHE = {}


def _build_nc(mode: str, zero_fill: bool):
    from concourse import bass, mybir

    QPX, NC_, NSL, SPAN, CALLS = _MODES[mode]
    nc = bass.Bass(target_bir_lowering=False)
    i32, f32 = mybir.dt.int32, mybir.dt.float32
    out = nc.dram_tensor("out", [OROWS, WPAD], f32, kind="ExternalOutput")
    idxs = nc.dram_tensor("idxs", [QPX, NC_], i32, kind="ExternalInput")
    kvals = nc.dram_tensor("kvals", [QPX, NSL * KS], f32, kind="ExternalInput")

    nfill = OROWS // 1344  # 33 zero-fill DMAs of [1344, 264] (contingency)

    with (
        nc.Block() as block,
        nc.semaphore("s_a") as s_a,
        nc.semaphore("s_b") as s_b,
        nc.semaphore("s_m") as s_m,
        nc.semaphore("s_f") as s_f,
        nc.semaphore("s_d") as s_d,
        nc.sbuf_tensor("sb_idx", [QPX, NC_], i32) as sb_idx,
        nc.sbuf_tensor("pbuf", [QPX, SPAN], f32) as pbuf,
        nc.sbuf_tensor("zt", [128, 2772], f32) as zt,
    ):

        @block.sync
        def _(sync):
            sync.dma_start(out=sb_idx[:], in_=idxs[:]).then_inc(s_a, 16)
            if zero_fill:
                sync.wait_ge(s_m, 2)
                for i in range(nfill):
                    sync.dma_start(
                        out=out[i * 1344:(i + 1) * 1344, :], in_=zt[:, :]
                    ).then_inc(s_f, 16)

        @block.scalar
        def _(scalar):
            # patch rows -> their span slots (strided dst, 36 B chunks)
            scalar.dma_start(
                out=bass.AP(pbuf, 0, [[SPAN, QPX], [WPAD, NSL], [1, KS]]),
                in_=kvals[:],
            ).then_inc(s_b, 16)

        @block.vector
        def _(vector):
            if zero_fill:
                vector.memset(zt[:], 0.0).then_inc(s_m, 1)
            # zero the inter-row gaps of the spans (disjoint from the
            # slot DMA above, so no ordering between them is needed)
            vector.memset(
                bass.AP(pbuf, KS, [[SPAN, QPX], [WPAD, NSL - 1], [1, WPAD - KS]]),
                0.0,
            ).then_inc(s_m, 1)

        @block.gpsimd
        def _(g):
            g.wait_ge(s_a, 16)
            g.wait_ge(s_b, 16)
            g.wait_ge(s_m, 2 if zero_fill else 1)
            if zero_fill:
                g.wait_ge(s_f, 16 * nfill)
            for np_, col in CALLS:
                g.indirect_dma_start(
                    out=out[:],
                    out_offset=bass.IndirectOffsetOnAxis(
                        ap=sb_idx[:np_, col:col + 1], axis=1
                    ),
                    in_=pbuf[:np_, :],
                    in_offset=None,
                ).then_inc(s_d, 16)
            g.wait_ge(s_d, 16 * len(CALLS))

    return nc


def _get_nc(mode: str, zero_fill: bool):
    key = (mode, zero_fill)
    if key not in _NC_CACHE:
        if mode in _MODES:
            nc = _build_nc(mode, zero_fill)
        else:
            nc = _build_nc_legacy(mode, zero_fill)
        if not nc.is_finalized():
            nc.finalize()
        _NC_CACHE[key] = nc
    return _NC_CACHE[key]


import os

_MODE = os.environ.get("HEATMAP_MODE", "spans2")


def _prep(x, kernel2d, mode):
    """Host prep: per-core span-start indices + shared pre-scaled values."""
    QPX, NC_, NSL, SPAN, CALLS = _MODES[mode]
    x = np.asarray(x)
    flip = np.asarray(kernel2d, dtype=np.float32)[::-1, ::-1]
    vals10 = (10.0 * flip).astype(np.float32)

    kv = np.zeros((QPX, NSL * KS), np.float32)
    if mode == "spans2":
        kv[:] = vals10.reshape(1, 81)
    else:  # rows3: partition q holds rows 3*(q%3) .. 3*(q%3)+2
        s = np.arange(QPX) % 3
        kv[:] = vals10.reshape(3, 27)[s]

    xr = x.reshape(NCORES, NPTS, 2)
    maps = []
    for core in range(NCORES):
        r = xr[core, :, 0].astype(np.int64)
        c = xr[core, :, 1].astype(np.int64)
        p = np.arange(NPTS)
        idx = np.zeros((QPX, NC_), np.int32)
        if mode == "spans2":
            start = (WPAD * (HP * p + r) + c).astype(np.int32)
            idx[:128, 0] = start[:128]
            idx[:40, 1] = start[128:]
        else:  # rows3: run 126*k + q <-> patch (126k+q)//3, span (q%3)
            run = np.arange(4 * QPX)
            rp, s = run // 3, run % 3
            start = (WPAD * (HP * rp + r[rp] + 3 * s) + c[rp]).astype(np.int32)
            idx[:, :] = start.reshape(4, QPX).T
        maps.append({"idxs": idx, "kvals": kv})
    return mode, maps


def _in_maps(x, kernel2d):
    return _prep(x, kernel2d, _MODE)


def _assemble(results):
    full = np.empty((B, KP, H, H), np.float32)
    for core, res in enumerate(results):
        o = res["out"].reshape(BLOC, KP, HP, WPAD)
        full[core * BLOC:(core + 1) * BLOC] = o[:, :, PAD:PAD + H, PAD:PAD + H]
    return full


def _run(mode, zero_fill, maps, **kw):
    from concourse.bass_utils import run_bass_kernel_spmd

    nc = _get_nc(mode, zero_fill)
    return run_bass_kernel_spmd(nc, maps, core_ids=list(range(NCORES)), **kw)


def _zero_contract_ok(x, results):
    """Sample must-be-zero cells to confirm outputs arrived pre-zeroed."""
    x = np.asarray(x).reshape(NCORES, NPTS, 2)
    rng = np.random.RandomState(0)
    for core in (0, NCORES - 1):
        o = results[core]["out"].reshape(NPTS, HP, WPAD)
        for p in rng.choice(NPTS, 24, replace=False):
            r = x[core, p, 0]
            rows = np.arange(HP)
            far = rows[(rows < r - 1) | (rows > r + KS + 1)]
            sel = rng.choice(far, 8, replace=False)
            if np.any(o[p][sel] != 0.0):
                return False
    return True


def _patches_ok(x, kernel2d, results):
    """Sample patches to confirm every span landed at the right address."""
    x = np.asarray(x).reshape(NCORES, NPTS, 2)
    vals10 = 10.0 * np.asarray(kernel2d, np.float32)[::-1, ::-1]
    rng = np.random.RandomState(1)
    for core in (0, NCORES // 2, NCORES - 1):
        o = results[core]["out"].reshape(NPTS, HP, WPAD)
        for p in rng.choice(NPTS, 16, replace=False):
            r, c = int(x[core, p, 0]), int(x[core, p, 1])
            got = o[p][r:r + KS, c:c + KS]
            if not np.allclose(got, vals10, rtol=1e-6, atol=1e-6):
                return False
    return True


def kernel(x, kernel2d):
    mode, maps = _in_maps(x, kernel2d)
    res = _run(mode, False, maps)
    if not _zero_contract_ok(x, res.results):
        # pre-zeroed-output contract failed; redo with explicit zero fill
        res = _run(mode, True, maps)
    if _patches_ok(x, kernel2d, res.results):
        return _assemble(res.results)
    # span scatter misbehaved on this HW: fall back to the
    # hardware-validated whole-patch-span kernel from the prior session
    return _legacy_kernel(x, kernel2d)


# ---------------------------------------------------------------------------
# Legacy fallback (hardware-validated previous-session kernel): whole-patch
# contiguous spans in a row-padded-only [NPTS*H + dump, 264] layout.
# ---------------------------------------------------------------------------

QP = 126
L_ROWS = NPTS * H           # 43008 image rows per core
L_PATCH = 8 * WPAD + KS     # 2121: contiguous span of one unclipped patch
L_NPDUMP = 16
L_DROWS = (L_NPDUMP * L_PATCH + QP * KS + WPAD - 1) // WPAD + 1
L_OROWS = L_ROWS + L_DROWS
L_DUMP = L_ROWS * WPAD
L_RDUMP = L_DUMP + L_NPDUMP * L_PATCH

_Q = np.arange(QP)
_T = _Q % KS
_P = 14 * np.arange(12)[None, :] + (_Q // KS)[:, None]   # [126,12] point id


def _build_nc_legacy(mode: str, zero_fill: bool):
    from concourse import bass, bacc, mybir
    import concourse.tile as tile

    nc = bacc.Bacc(None, target_bir_lowering=False)
    i32, f32 = mybir.dt.int32, mybir.dt.float32
    out = nc.dram_tensor("out", [L_OROWS, WPAD], f32, kind="ExternalOutput")

    if mode == "patch3":
        idxs = nc.dram_tensor("idxs", [QP, 3], i32, kind="ExternalInput")
        kvals = nc.dram_tensor("kvals", [QP, 90], f32, kind="ExternalInput")
    else:  # rows12: one 9-elem segment per patch-row slot
        idxs = nc.dram_tensor("idxs", [QP, 12], i32, kind="ExternalInput")
        kvals = nc.dram_tensor("kvals", [QP, 108], f32, kind="ExternalInput")

    with tile.TileContext(nc) as tc:
        with tc.tile_pool(name="sbuf", bufs=1) as pool:
            if mode == "patch3":
                idx_t = pool.tile([QP, 3], i32)
                kv_t = pool.tile([QP, 90], f32)
            else:
                idx_t = pool.tile([QP, 12], i32)
                kv_t = pool.tile([QP, 108], f32)
            if mode == "patch3":
                pbuf = pool.tile([QP, L_PATCH], f32)
                nc.vector.memset(pbuf[:], 0.0)
            nc.sync.dma_start(out=idx_t[:], in_=idxs[:])
            nc.sync.dma_start(out=kv_t[:], in_=kvals[:])

            if zero_fill:
                zt = pool.tile([128, 2772], mybir.dt.float32)
                nc.vector.memset(zt[:], 0.0)
                blk = 1344
                for i in range(L_ROWS // blk):
                    nc.sync.dma_start(
                        out=out[i * blk:(i + 1) * blk, :], in_=zt[:, :]
                    )
                nc.sync.dma_start(
                    out=out[L_ROWS:L_ROWS + 128, :], in_=zt[:, :WPAD]
                )
                nc.sync.dma_start(
                    out=out[L_ROWS + 128:L_OROWS, :],
                    in_=zt[:L_DROWS - 128, :WPAD],
                )

            if mode == "patch3":
                rbuf = pool.tile([QP, KS], f32)
                for k in range(KS):
                    nc.vector.tensor_scalar_mul(
                        pbuf[:, k * WPAD:k * WPAD + KS],
                        kv_t[:, k * KS:(k + 1) * KS],
                        10.0,
                    )
                nc.vector.tensor_scalar_mul(rbuf[:], kv_t[:, 81:90], 10.0)
                for ap_in, ap_idx in (
                    (pbuf[:], idx_t[:, 0:1]),
                    (pbuf[:42, :], idx_t[:42, 1:2]),
                    (rbuf[:], idx_t[:, 2:3]),
                ):
                    nc.gpsimd.indirect_dma_start(
                        out=out[:],
                        out_offset=bass.IndirectOffsetOnAxis(ap=ap_idx, axis=1),
                        in_=ap_in,
                        in_offset=None,
                    )
            else:
                k10 = pool.tile([QP, 12, KS], f32)
                nc.vector.tensor_scalar_mul(k10[:], kv_t[:], 10.0)
                for j in range(12):
                    nc.gpsimd.indirect_dma_start(
                        out=out[:],
                        out_offset=bass.IndirectOffsetOnAxis(
                            ap=idx_t[:, j:j + 1], axis=1
                        ),
                        in_=k10[:, j, :],
                        in_offset=None,
                    )
    return nc


def _legacy_prep_patch3(xc, flip):
    idxs = np.empty((QP, 3), np.int32)
    idxs[:, 0] = L_DUMP + (np.arange(QP) % L_NPDUMP) * L_PATCH
    idxs[:, 1] = L_DUMP + (np.arange(QP) % L_NPDUMP) * L_PATCH
    idxs[:, 2] = L_RDUMP + np.arange(QP) * KS
    kvals = np.zeros((QP, 90), np.float32)
    kvals[:, :81] = flip.reshape(-1)[None, :]
    clip_i = []
    clip_k = []
    ndump = 0
    for p in range(NPTS):
        r, c = int(xc[p, 0]), int(xc[p, 1])
        start = WPAD * (H * p + r - PAD) + c
        if PAD <= r <= H - 1 - PAD:
            if p < QP:
                idxs[p, 0] = start
            else:
                idxs[p - QP, 1] = start
        else:
            ndump += 1
            for t in range(KS):
                rp = r - PAD + t
                if 0 <= rp < H:
                    clip_i.append(WPAD * (H * p + rp) + c)
                    clip_k.append(flip[t])
    if len(clip_i) > QP or ndump > L_NPDUMP:
        return None
    if clip_i:
        idxs[: len(clip_i), 2] = clip_i
        kvals[: len(clip_k), 81:90] = clip_k
    return idxs, kvals


def _legacy_prep_rows12(xc, flip):
    r = xc[_P, 0].astype(np.int64)
    c = xc[_P, 1].astype(np.int64)
    rp = r + _T[:, None] - PAD
    sidx = WPAD * (H * _P + rp) + c
    slot = (_Q[:, None] * 12 + np.arange(12)[None, :]) % (QP * 12)
    dump = L_DUMP + (slot % ((L_DROWS * WPAD) // KS - 1)) * KS
    sidx = np.where((rp < 0) | (rp >= H), dump, sidx).astype(np.int32)
    kvals = np.ascontiguousarray(
        np.broadcast_to(flip[_T][:, None, :], (QP, 12, KS))
    ).reshape(QP, 108).astype(np.float32)
    return sidx, kvals


def _legacy_assemble(results):
    full = np.empty((B, KP, H, H), np.float32)
    for core, res in enumerate(results):
        o = res["out"][:L_ROWS].reshape(BLOC, KP, H, WPAD)
        full[core * BLOC:(core + 1) * BLOC] = o[:, :, :, PAD:PAD + H]
    return full


def _legacy_zero_ok(x, results):
    x = np.asarray(x).reshape(NCORES, NPTS, 2)
    rng = np.random.RandomState(0)
    for core in (0, NCORES - 1):
        o = results[core]["out"][:L_ROWS].reshape(NPTS, H, WPAD)
        for p in rng.choice(NPTS, 24, replace=False):
            r = x[core, p, 0]
            rows = np.arange(H)
            far = rows[(rows < r - PAD - 1) | (rows > r + PAD + 1)]
            sel = rng.choice(far, 8, replace=False)
            if np.any(o[p][sel] != 0.0):
                return False
    return True


def _legacy_kernel(x, kernel2d):
    x = np.asarray(x)
    flip = np.asarray(kernel2d, dtype=np.float32)[::-1, ::-1]
    xr = x.reshape(NCORES, NPTS, 2)
    preps = [_legacy_prep_patch3(xr[c], flip) for c in range(NCORES)]
    if all(p is not None for p in preps):
        mode = "patch3"
        maps = [{"idxs": p[0], "kvals": p[1]} for p in preps]
    else:
        mode = "rows12"
        maps = []
        for core in range(NCORES):
            sidx, kvals = _legacy_prep_rows12(xr[core], flip)
            maps.append({"idxs": sidx, "kvals": kvals})
    res = _run(mode, False, maps)
    if not _legacy_zero_ok(x, res.results):
        res = _run(mode, True, maps)
    return _legacy_assemble(res.results)


# revision 6
# speedup vs baseline: 1.2150x; 1.2150x over previous
"""Trainium2 Bass kernel for nn_HeatmapBatch.

Reference computes: one-hot delta (value 10.0) per (batch, keypoint) at
integer coords (r, c) in a 256x256 image, then depthwise-convolves with a
shared 9x9 kernel.  Since each image holds exactly one delta, the output is
zeros everywhere except a 9x9 patch of 10*kernel2d[::-1,::-1] (XLA conv is
cross-correlation) centred at (r, c), clipped at the borders.

Device strategy (data-parallel over batch, 8 cores x 8 batches = 168
images per core):

  Each image gets a fully padded 264x264 canvas (4 pad rows/cols on every
  side), so every patch -- clipped or not -- is in-bounds: no clip
  handling, no dump zone (the host strips the padding on assemble).

  The gpsimd indirect-DMA scatter writes, per SBUF partition, ONE
  contiguous run (= the partition's whole source payload) at that
  partition's index (hardware-verified semantics; extra offset-AP columns
  are ignored for addressing).  A patch is therefore written as a small
  number of contiguous multi-row spans whose inter-row gaps are zeros
  (overwriting pad/zero cells with zeros is harmless):

    - mode "spans2": whole 9-row spans (2121 elems), 2 calls (128+40
      partitions), 1.43 MB written/core.
    - mode "rows3": 3-row spans (537 elems), 4 calls x 126 partitions
      (run s of patch p sits at partition q with q%3 == s, so one shared
      [126, 537] value buffer serves all 4 calls), 1.08 MB written/core.

  The value buffer is composed without any vector-compute stage: host
  pre-scales (10*flip), a strided sync-DMA drops the rows into their
  span slots, and a vector memset zeroes the gaps (disjoint regions, no
  ordering).  gpsimd issues the index DMA itself (SWDGE) to skip one
  cross-engine handoff.  The runtime hands kernels pre-zeroed
  ExternalOutput buffers (documented contract in bass_utils/bass2jax),
  so the kernel only scatters; a zero-fill variant covers the contract
  ever failing (detected by output sampling), and the previous session's
  hardware-validated kernel remains as a final fallback.
Host does sharding/layout prep and the final gather/strip of the padding.
"""

import numpy as np


def _ensure_axon_hooks():
    """bass_utils imports antenv.axon_hooks when tracing is requested (e.g.
    BASS_TRACE=1 in the environment); some images lack that module.  Provide
    it best-effort so a tracing harness degrades gracefully instead of
    crashing.  Never raises."""
    try:
        import antenv.axon_hooks  # noqa: F401
        return
    except Exception:
        pass
    try:
        import sys
        import types

        import antenv

        mod = types.ModuleType("antenv.axon_hooks")
        _state = {"hook": None}
        mod.set_axon_ntff_profile_hook = lambda h: _state.__setitem__("hook", h)
        mod.get_axon_ntff_profile_hook = lambda: _state["hook"]
        sys.modules["antenv.axon_hooks"] = mod
        antenv.axon_hooks = mod
        try:
            from trn_agent_boot.trn_boot import _ntff_profile_via_ctypes

            mod.set_axon_ntff_profile_hook(
                _ntff_profile_via_ctypes("/opt/axon/libaxon_pjrt.so")
            )
        except Exception:
            pass
    except Exception:
        pass


_ensure_axon_hooks()

B, KP, H = 64, 21, 256
KS, PAD = 9, 4
NCORES = 8
BLOC = B // NCORES          # 8 batches per core
NPTS = BLOC * KP            # 168 images per core
WPAD = H + 2 * PAD          # 264 padded columns
HP = H + 2 * PAD            # 264 padded rows per image
OROWS = NPTS * HP           # 44352 output rows per core

# mode -> (partitions, idx cols, rows per span, span length, calls)
# calls: list of (n_partitions, idx column) per indirect DMA
_MODES = {
    "spans2": (128, 2, 9, 8 * WPAD + KS, [(128, 0), (40, 1)]),
    "rows3": (126, 4, 3, 2 * WPAD + KS, [(126, 0), (126, 1), (126, 2), (126, 3)]),
}

_NC_CACHE = {}


def _build_nc(mode: str, zero_fill: bool):
    from concourse import bass, mybir

    QPX, NC_, NSL, SPAN, CALLS = _MODES[mode]
    nc = bass.Bass(target_bir_lowering=False)
    i32 = mybir.dt.int32
    fdt = mybir.dt.float16 if _HALF else mybir.dt.float32
    out = nc.dram_tensor("out", [OROWS, WPAD], fdt, kind="ExternalOutput")
    idxs = nc.dram_tensor("idxs", [QPX, NC_], i32, kind="ExternalInput")
    kvals = nc.dram_tensor("kvals", [QPX, NSL * KS], fdt, kind="ExternalInput")

    nfill = OROWS // 1344  # 33 zero-fill DMAs of [1344, 264] (contingency)

    with (
        nc.Block() as block,
        nc.semaphore("s_a") as s_a,
        nc.semaphore("s_b") as s_b,
        nc.semaphore("s_m") as s_m,
        nc.semaphore("s_z") as s_z,
        nc.semaphore("s_f") as s_f,
        nc.semaphore("s_d") as s_d,
        nc.sbuf_tensor("sb_idx", [QPX, NC_], i32) as sb_idx,
        nc.sbuf_tensor("kv_t", [QPX, NSL * KS], fdt) as kv_t,
        nc.sbuf_tensor("pbuf", [QPX, SPAN], fdt) as pbuf,
        nc.sbuf_tensor("zt", [128, 2772], fdt) as zt,
    ):

        @block.sync
        def _(sync):
            sync.dma_start(out=sb_idx[:], in_=idxs[:]).then_inc(s_a, 16)
            if zero_fill:
                sync.wait_ge(s_z, 1)
                for i in range(nfill):
                    sync.dma_start(
                        out=out[i * 1344:(i + 1) * 1344, :], in_=zt[:, :]
                    ).then_inc(s_f, 16)

        @block.scalar
        def _(scalar):
            scalar.dma_start(out=kv_t[:], in_=kvals[:]).then_inc(s_b, 16)

        @block.vector
        def _(vector):
            if zero_fill:
                vector.memset(zt[:], 0.0).then_inc(s_z, 1)
            vector.memset(pbuf[:], 0.0).then_inc(s_m, 1)

        @block.gpsimd
        def _(g):
            # pbuf is written by exactly one engine at a time (vector
            # memset, then this strided slot copy) -- concurrent DVE/DMA
            # writers to one SBUF row can drop sub-word updates on fp16
            g.wait_ge(s_b, 16)
            g.wait_ge(s_m, 1)
            g.tensor_scalar_mul(
                bass.AP(pbuf, 0, [[SPAN, QPX], [WPAD, NSL], [1, KS]]),
                kv_t[:],
                1.0,
            )
            g.wait_ge(s_a, 16)
            if zero_fill:
                g.wait_ge(s_f, 16 * nfill)
            for np_, col in CALLS:
                g.indirect_dma_start(
                    out=out[:],
                    out_offset=bass.IndirectOffsetOnAxis(
                        ap=sb_idx[:np_, col:col + 1], axis=1
                    ),
                    in_=pbuf[:np_, :],
                    in_offset=None,
                ).then_inc(s_d, 16)
            g.wait_ge(s_d, 16 * len(CALLS))

    return nc


def _get_nc(mode: str, zero_fill: bool):
    key = (mode, zero_fill)
    if key not in _NC_CACHE:
        if mode in _MODES:
            nc = _build_nc(mode, zero_fill)
        else:
            nc = _build_nc_legacy(mode, zero_fill)
        if not nc.is_finalized():
            nc.finalize()
        _NC_CACHE[key] = nc
    return _NC_CACHE[key]


import os

_MODE = os.environ.get("HEATMAP_MODE", "spans2")
_HALF = os.environ.get("HEATMAP_DT", "f16") == "f16"
_ODT = np.float16 if _HALF else np.float32


def _prep(x, kernel2d, mode):
    """Host prep: per-core span-start indices + shared pre-scaled values."""
    QPX, NC_, NSL, SPAN, CALLS = _MODES[mode]
    x = np.asarray(x)
    flip = np.asarray(kernel2d, dtype=np.float32)[::-1, ::-1]
    vals10 = (10.0 * flip).astype(np.float32)

    kv = np.zeros((QPX, NSL * KS), _ODT)
    if mode == "spans2":
        kv[:] = vals10.reshape(1, 81)
    else:  # rows3: partition q holds rows 3*(q%3) .. 3*(q%3)+2
        s = np.arange(QPX) % 3
        kv[:] = vals10.reshape(3, 27)[s]

    xr = x.reshape(NCORES, NPTS, 2)
    maps = []
    for core in range(NCORES):
        r = xr[core, :, 0].astype(np.int64)
        c = xr[core, :, 1].astype(np.int64)
        p = np.arange(NPTS)
        idx = np.zeros((QPX, NC_), np.int32)
        if mode == "spans2":
            start = (WPAD * (HP * p + r) + c).astype(np.int32)
            idx[:128, 0] = start[:128]
            idx[:40, 1] = start[128:]
        else:  # rows3: run 126*k + q <-> patch (126k+q)//3, span (q%3)
            run = np.arange(4 * QPX)
            rp, s = run // 3, run % 3
            start = (WPAD * (HP * rp + r[rp] + 3 * s) + c[rp]).astype(np.int32)
            idx[:, :] = start.reshape(4, QPX).T
        maps.append({"idxs": idx, "kvals": kv})
    return mode, maps


def _in_maps(x, kernel2d):
    return _prep(x, kernel2d, _MODE)


def _assemble(results):
    full = np.empty((B, KP, H, H), np.float32)
    for core, res in enumerate(results):
        o = res["out"].reshape(BLOC, KP, HP, WPAD)
        full[core * BLOC:(core + 1) * BLOC] = o[
            :, :, PAD:PAD + H, PAD:PAD + H
        ].astype(np.float32)
    return full


def _run(mode, zero_fill, maps, **kw):
    from concourse.bass_utils import run_bass_kernel_spmd

    nc = _get_nc(mode, zero_fill)
    return run_bass_kernel_spmd(nc, maps, core_ids=list(range(NCORES)), **kw)


def _zero_contract_ok(x, results):
    """Sample must-be-zero cells to confirm outputs arrived pre-zeroed."""
    x = np.asarray(x).reshape(NCORES, NPTS, 2)
    rng = np.random.RandomState(0)
    for core in (0, NCORES - 1):
        o = results[core]["out"].reshape(NPTS, HP, WPAD)
        for p in rng.choice(NPTS, 24, replace=False):
            r = x[core, p, 0]
            rows = np.arange(HP)
            far = rows[(rows < r - 1) | (rows > r + KS + 1)]
            sel = rng.choice(far, 8, replace=False)
            if np.any(o[p][sel] != 0.0):
                return False
    return True


def _patches_ok(x, kernel2d, results):
    """Sample patches to confirm every span landed at the right address."""
    x = np.asarray(x).reshape(NCORES, NPTS, 2)
    vals10 = 10.0 * np.asarray(kernel2d, np.float32)[::-1, ::-1]
    rng = np.random.RandomState(1)
    for core in (0, NCORES // 2, NCORES - 1):
        o = results[core]["out"].reshape(NPTS, HP, WPAD)
        for p in rng.choice(NPTS, 16, replace=False):
            r, c = int(x[core, p, 0]), int(x[core, p, 1])
            got = np.asarray(o[p][r:r + KS, c:c + KS], np.float32)
            tol = 0.02 if _HALF else 1e-6
            if not np.allclose(got, vals10, rtol=0.0, atol=tol):
                return False
    return True


def kernel(x, kernel2d):
    mode, maps = _in_maps(x, kernel2d)
    res = _run(mode, False, maps)
    if not _zero_contract_ok(x, res.results):
        # pre-zeroed-output contract failed; redo with explicit zero fill
        res = _run(mode, True, maps)
    if _patches_ok(x, kernel2d, res.results):
        return _assemble(res.results)
    # span scatter misbehaved on this HW: fall back to the
    # hardware-validated whole-patch-span kernel from the prior session
    return _legacy_kernel(x, kernel2d)


# ---------------------------------------------------------------------------
# Legacy fallback (hardware-validated previous-session kernel): whole-patch
# contiguous spans in a row-padded-only [NPTS*H + dump, 264] layout.
# ---------------------------------------------------------------------------

QP = 126
L_ROWS = NPTS * H           # 43008 image rows per core
L_PATCH = 8 * WPAD + KS     # 2121: contiguous span of one unclipped patch
L_NPDUMP = 16
L_DROWS = (L_NPDUMP * L_PATCH + QP * KS + WPAD - 1) // WPAD + 1
L_OROWS = L_ROWS + L_DROWS
L_DUMP = L_ROWS * WPAD
L_RDUMP = L_DUMP + L_NPDUMP * L_PATCH

_Q = np.arange(QP)
_T = _Q % KS
_P = 14 * np.arange(12)[None, :] + (_Q // KS)[:, None]   # [126,12] point id


def _build_nc_legacy(mode: str, zero_fill: bool):
    from concourse import bass, bacc, mybir
    import concourse.tile as tile

    nc = bacc.Bacc(None, target_bir_lowering=False)
    i32, f32 = mybir.dt.int32, mybir.dt.float32
    out = nc.dram_tensor("out", [L_OROWS, WPAD], f32, kind="ExternalOutput")

    if mode == "patch3":
        idxs = nc.dram_tensor("idxs", [QP, 3], i32, kind="ExternalInput")
        kvals = nc.dram_tensor("kvals", [QP, 90], f32, kind="ExternalInput")
    else:  # rows12: one 9-elem segment per patch-row slot
        idxs = nc.dram_tensor("idxs", [QP, 12], i32, kind="ExternalInput")
        kvals = nc.dram_tensor("kvals", [QP, 108], f32, kind="ExternalInput")

    with tile.TileContext(nc) as tc:
        with tc.tile_pool(name="sbuf", bufs=1) as pool:
            if mode == "patch3":
                idx_t = pool.tile([QP, 3], i32)
                kv_t = pool.tile([QP, 90], f32)
            else:
                idx_t = pool.tile([QP, 12], i32)
                kv_t = pool.tile([QP, 108], f32)
            if mode == "patch3":
                pbuf = pool.tile([QP, L_PATCH], f32)
                nc.vector.memset(pbuf[:], 0.0)
            nc.sync.dma_start(out=idx_t[:], in_=idxs[:])
            nc.sync.dma_start(out=kv_t[:], in_=kvals[:])

            if zero_fill:
                zt = pool.tile([128, 2772], mybir.dt.float32)
                nc.vector.memset(zt[:], 0.0)
                blk = 1344
                for i in range(L_ROWS // blk):
                    nc.sync.dma_start(
                        out=out[i * blk:(i + 1) * blk, :], in_=zt[:, :]
                    )
                nc.sync.dma_start(
                    out=out[L_ROWS:L_ROWS + 128, :], in_=zt[:, :WPAD]
                )
                nc.sync.dma_start(
                    out=out[L_ROWS + 128:L_OROWS, :],
                    in_=zt[:L_DROWS - 128, :WPAD],
                )

            if mode == "patch3":
                rbuf = pool.tile([QP, KS], f32)
                for k in range(KS):
                    nc.vector.tensor_scalar_mul(
                        pbuf[:, k * WPAD:k * WPAD + KS],
                        kv_t[:, k * KS:(k + 1) * KS],
                        10.0,
                    )
                nc.vector.tensor_scalar_mul(rbuf[:], kv_t[:, 81:90], 10.0)
                for ap_in, ap_idx in (
                    (pbuf[:], idx_t[:, 0:1]),
                    (pbuf[:42, :], idx_t[:42, 1:2]),
                    (rbuf[:], idx_t[:, 2:3]),
                ):
                    nc.gpsimd.indirect_dma_start(
                        out=out[:],
                        out_offset=bass.IndirectOffsetOnAxis(ap=ap_idx, axis=1),
                        in_=ap_in,
                        in_offset=None,
                    )
            else:
                k10 = pool.tile([QP, 12, KS], f32)
                nc.vector.tensor_scalar_mul(k10[:], kv_t[:], 10.0)
                for j in range(12):
                    nc.gpsimd.indirect_dma_start(
                        out=out[:],
                        out_offset=bass.IndirectOffsetOnAxis(
                            ap=idx_t[:, j:j + 1], axis=1
                        ),
                        in_=k10[:, j, :],
                        in_offset=None,
                    )
    return nc


def _legacy_prep_patch3(xc, flip):
    idxs = np.empty((QP, 3), np.int32)
    idxs[:, 0] = L_DUMP + (np.arange(QP) % L_NPDUMP) * L_PATCH
    idxs[:, 1] = L_DUMP + (np.arange(QP) % L_NPDUMP) * L_PATCH
    idxs[:, 2] = L_RDUMP + np.arange(QP) * KS
    kvals = np.zeros((QP, 90), np.float32)
    kvals[:, :81] = flip.reshape(-1)[None, :]
    clip_i = []
    clip_k = []
    ndump = 0
    for p in range(NPTS):
        r, c = int(xc[p, 0]), int(xc[p, 1])
        start = WPAD * (H * p + r - PAD) + c
        if PAD <= r <= H - 1 - PAD:
            if p < QP:
                idxs[p, 0] = start
            else:
                idxs[p - QP, 1] = start
        else:
            ndump += 1
            for t in range(KS):
                rp = r - PAD + t
                if 0 <= rp < H:
                    clip_i.append(WPAD * (H * p + rp) + c)
                    clip_k.append(flip[t])
    if len(clip_i) > QP or ndump > L_NPDUMP:
        return None
    if clip_i:
        idxs[: len(clip_i), 2] = clip_i
        kvals[: len(clip_k), 81:90] = clip_k
    return idxs, kvals


def _legacy_prep_rows12(xc, flip):
    r = xc[_P, 0].astype(np.int64)
    c = xc[_P, 1].astype(np.int64)
    rp = r + _T[:, None] - PAD
    sidx = WPAD * (H * _P + rp) + c
    slot = (_Q[:, None] * 12 + np.arange(12)[None, :]) % (QP * 12)
    dump = L_DUMP + (slot % ((L_DROWS * WPAD) // KS - 1)) * KS
    sidx = np.where((rp < 0) | (rp >= H), dump, sidx).astype(np.int32)
    kvals = np.ascontiguousarray(
        np.broadcast_to(flip[_T][:, None, :], (QP, 12, KS))
    ).reshape(QP, 108).astype(np.float32)
    return sidx, kvals


def _legacy_assemble(results):
    full = np.empty((B, KP, H, H), np.float32)
    for core, res in enumerate(results):
        o = res["out"][:L_ROWS].reshape(BLOC, KP, H, WPAD)
        full[core * BLOC:(core + 1) * BLOC] = o[:, :, :, PAD:PAD + H]
    return full


def _legacy_zero_ok(x, results):
    x = np.asarray(x).reshape(NCORES, NPTS, 2)
    rng = np.random.RandomState(0)
    for core in (0, NCORES - 1):
        o = results[core]["out"][:L_ROWS].reshape(NPTS, H, WPAD)
        for p in rng.choice(NPTS, 24, replace=False):
            r = x[core, p, 0]
            rows = np.arange(H)
            far = rows[(rows < r - PAD - 1) | (rows > r + PAD + 1)]
            sel = rng.choice(far, 8, replace=False)
            if np.any(o[p][sel] != 0.0):
                return False
    return True


def _legacy_kernel(x, kernel2d):
    x = np.asarray(x)
    flip = np.asarray(kernel2d, dtype=np.float32)[::-1, ::-1]
    xr = x.reshape(NCORES, NPTS, 2)
    preps = [_legacy_prep_patch3(xr[c], flip) for c in range(NCORES)]
    if all(p is not None for p in preps):
        mode = "patch3"
        maps = [{"idxs": p[0], "kvals": p[1]} for p in preps]
    else:
        mode = "rows12"
        maps = []
        for core in range(NCORES):
            sidx, kvals = _legacy_prep_rows12(xr[core], flip)
            maps.append({"idxs": sidx, "kvals": kvals})
    res = _run(mode, False, maps)
    if not _legacy_zero_ok(x, res.results):
        res = _run(mode, True, maps)
    return _legacy_assemble(res.results)
